# Initial kernel scaffold
#
"""Trainium2 Bass kernel for the B-spline (KAN-style) layer.

Math: out[b,o] = sum_{i,k} B3_k(t(b,i)) * coeff[i,o,k], where t = tanh(x)
mapped to knot coordinates t = (tanh(x) - grid[0]) / h in (3, 8), and B3 are
cubic B-spline bases over uniform integer knots.

Key transformation: each cubic B-spline basis is an exact linear combination
of truncated cubes L_m = relu(t-m)^3 and R_m = relu(m-t)^3, m in {4,5,6,7}
(divided-difference / truncated-power identity, binomial weights [1,-4,6,-4,1]/6).
The 8x8 basis-change matrix is folded into the coefficient tensor on the host,
so the device only computes 8 "plane" tensors per input feature:
    w_m = 2.5*tanh(x) + (5.5 - m)   (DVE tensor_scalar)
    c_m = w_m^3                     (DVE: two tensor_mul)
    L_m = max(c_m, 0) ; Rn_m = min(c_m, 0) = -R_m   (DVE tensor_scalar max/min)
(the -R sign and the plane ordering are folded into the host coefficients).

Then a dense fp16 matmul: out[o,b] = sum_{(i,plane)} C3[(i,plane),o] * rho[(i,plane),b]
with contraction K = 512*8 = 4096, run on the PE at full bf16/fp16 rate.

Sharding: data-parallel over batch (8192 -> 8 x 1024); coefficients replicated.
Inputs are transposed on the host so the feature dim i lands on SBUF partitions
(making the contraction dim the partition dim for the matmul with no on-device
transposes); the output is produced as (o, b) and transposed back on the host.
"""

from contextlib import ExitStack

import numpy as np

import concourse.bass as bass
import concourse.mybir as mybir
import concourse.tile as tile
from concourse.bass_utils import run_bass_kernel_spmd
from concourse.vector_clock import ScopedClock

F32 = mybir.dt.float32
F16 = mybir.dt.float16

N_CORES = 8
B_FULL = 8192
B_SHARD = B_FULL // N_CORES  # 1024
I_FEAT = 512
O_FEAT = 512
NPLANES = 8
NCHUNK = I_FEAT // 128  # 4
ALU = mybir.AluOpType
AF = mybir.ActivationFunctionType

# ---------------------------------------------------------------------------
# Workaround for walrus "Too many sync wait commands" on the TileContext final
# Drain: spread the accumulated semaphore waits across single-wait nofuse NOPs
# on the sync engine, then emit a bare drain + the usual barrier/cleanup.
_MAXW = 1


def _patched_drain_and_barrier(self, tick_clock, wait_clock):
    nc = self.nc
    probe = nc.sync.nop(nofuse=True)
    wait_clock.add_sem_waits(probe.ins, ScopedClock({None: tick_clock.global_clock}))
    si = probe.ins.sync_info
    waits = list(si.on_wait) if si and si.on_wait else []
    if len(waits) > _MAXW:
        si.on_wait = waits[:_MAXW]
        rest = waits[_MAXW:]
        while rest:
            chunk, rest = rest[:_MAXW], rest[_MAXW:]
            n2 = nc.sync.nop(nofuse=True)
            s2 = n2.ins.sync_info
            if s2 is None:
                n2.ins.sync_info = mybir.SyncInfo(on_wait=chunk, on_update=[])
            else:
                s2.on_wait = chunk
    nc.sync.drain()
    nc.all_engine_barrier()
    assert self.sems is not None
    popped = nc._tile_sem_poison_stack.pop()
    assert popped is self._sem_poison
    nc.clear_and_free_semaphores(list(self.sems.allocated().values()))


tile.TileContext._drain_and_barrier = _patched_drain_and_barrier


def _split_all_waits(nc: bass.Bass) -> None:
    """This image's walrus rejects instructions carrying more than one sync
    wait. Hoist all but the last wait of each instruction onto fresh NoOps on
    the same engine immediately before it (in-order issue makes this
    equivalent, merely slightly stronger synchronization)."""
    cnt = 0
    for f in nc.m.functions:
        for bb in f.blocks:
            out = []
            changed = False
            for inst in bb.instructions:
                si = inst.sync_info
                waits = list(si.on_wait) if si and si.on_wait else []
                if len(waits) > 1:
                    changed = True
                    for w in waits[:-1]:
                        nop = mybir.InstNoOp(name=f"waitsplit-{cnt}", ins=[], outs=[])
                        cnt += 1
                        nop.engine = inst.engine
                        nop.sync_info = mybir.SyncInfo(on_wait=[w], on_update=[])
                        out.append(nop)
                    si.on_wait = [waits[-1]]
                out.append(inst)
            if changed:
                bb.instructions = out


# ---------------------------------------------------------------------------


def _build_nc(t_scale: float, t_bias: float) -> bass.Bass:
    """Build the per-core Bass program.

    Per-core I/O:
      xt : (512, 1024) f16   x^T shard (feature-major)
      c3 : (4, 128, 4096) f16  folded coefficients [chunk, part, plane*512+o]
      out: (512, 1024) f32   output (o, b) shard
    """
    nc = bass.Bass()
    xt = nc.declare_dram_parameter("xt", [I_FEAT, B_SHARD], F16, isOutput=False)
    c3 = nc.declare_dram_parameter(
        "c3", [NCHUNK, 128, NPLANES * O_FEAT], F16, isOutput=False
    )
    out = nc.declare_dram_parameter("out", [O_FEAT, B_SHARD], F32, isOutput=True)

    with tile.TileContext(nc) as tc, ExitStack() as ctx:
        c3_pool = ctx.enter_context(tc.tile_pool(name="c3", bufs=1))
        xin_pool = ctx.enter_context(tc.tile_pool(name="xin", bufs=2))
        xn_pool = ctx.enter_context(tc.tile_pool(name="xn", bufs=2))
        sq_pool = ctx.enter_context(tc.tile_pool(name="sq", bufs=3))
        aff_pool = ctx.enter_context(tc.tile_pool(name="aff", bufs=3))
        rho_pool = ctx.enter_context(tc.tile_pool(name="rho", bufs=1))
        ps_pool = ctx.enter_context(
            tc.tile_pool(name="ps", bufs=1, space=bass.MemorySpace.PSUM)
        )
        ost_pool = ctx.enter_context(tc.tile_pool(name="ost", bufs=1))

        BHALF = B_SHARD // 2  # 512

        # c3 chunks on the scalar engine's HWDGE ring (qActDynamicHW) so they
        # stream in parallel with the latency-critical xt loads on the sync
        # ring (qSPDynamicHW). Triggers cost ~0.6us each on ACT, which has
        # slack.
        # Dummy 1-column activation with no deps: hoists the ~2.7us ACT
        # table load to kernel start, off the tanh critical path.
        dummy = xn_pool.tile([128, 1], F16, tag="dummy")
        nc.gpsimd.memset(dummy[:], 0.0)
        nc.scalar.activation(dummy[:], dummy[:], AF.Tanh)

        c3_sb = []
        for c in range(NCHUNK):
            ct = c3_pool.tile([128, NPLANES * O_FEAT], F16, tag=f"c3_{c}")
            nc.scalar.dma_start(ct[:], c3[c])
            c3_sb.append(ct)

        # Elementwise plane production at half-batch granularity (FD=512) so
        # the first matmul's input chain is short; b-half-major order matches
        # the matmul passes. rho[bh][c][r] planes are interleaved
        # [L4, -R4, L5, -R5, ...] (sign + order folded into c3).
        rho = [[[None] * NPLANES for _ in range(NCHUNK)] for _ in range(2)]
        for bh in range(2):
            for c in range(NCHUNK):
                xt_sb = xin_pool.tile([128, BHALF], F16, tag="xt")
                nc.sync.dma_start(
                    xt_sb[:],
                    xt[c * 128 : (c + 1) * 128, bh * BHALF : (bh + 1) * BHALF],
                )
                xn = xn_pool.tile([128, BHALF], F16, tag="xn")
                nc.scalar.activation(xn[:], xt_sb[:], AF.Tanh)
                from contextlib import nullcontext

                from concourse.tile import add_dep_helper

                prev_rp = None
                for mi, m in enumerate((4, 5, 6, 7)):
                    beta = t_bias - m
                    # The very first plane gates the whole matmul stream: pin
                    # its chain to priority 0 so the scheduler doesn't
                    # interleave the next m's ops into it (~0.6us).
                    prio = (
                        tc.high_priority()
                        if (bh == 0 and c == 0 and mi == 0)
                        else nullcontext()
                    )
                    with prio:
                        a = aff_pool.tile([128, BHALF], F16, tag="a")
                        a_inst = nc.vector.tensor_scalar(
                            a[:], xn[:], t_scale, beta, ALU.mult, ALU.add
                        )
                        if bh == 0 and c == 0 and mi == 1 and prev_rp is not None:
                            # Order-only edge: keep the scheduler from
                            # interleaving m5's ops into the m4 chain that
                            # gates the whole matmul stream (~0.45us).
                            add_dep_helper(
                                a_inst.ins,
                                prev_rp.ins,
                                sync=False,
                                reason="first-plane chain priority",
                            )
                        # Square and cube on DVE: keeps the whole plane chain
                        # on one strict-FIFO engine (the scheduler once moved
                        # next-chunk tanh ahead of ACT squares, stalling 5us).
                        s = sq_pool.tile([128, BHALF], F16, tag="s")
                        nc.vector.tensor_mul(s[:], a[:], a[:])
                        cc = sq_pool.tile([128, BHALF], F16, tag="cube")
                        nc.vector.tensor_mul(cc[:], s[:], a[:])
                        lp = rho_pool.tile([128, BHALF], F16, tag=f"rho{bh}_{c}_{mi}")
                        nc.vector.tensor_scalar_max(lp[:], cc[:], 0.0)
                        rp = rho_pool.tile(
                            [128, BHALF], F16, tag=f"rho{bh}_{c}_{mi + 4}"
                        )
                        prev_rp = nc.vector.tensor_scalar_min(rp[:], cc[:], 0.0)
                    # Planes interleaved [L4,R4,L5,R5,...] in the K order so
                    # the matmul consumes them exactly in production order
                    # (the matching column permutation is folded into c3).
                    rho[bh][c][2 * mi] = lp
                    rho[bh][c][2 * mi + 1] = rp

        # Dense matmul: 8 PSUM tiles (o_chunk x b_half) accumulated over all
        # 32 (chunk, plane) K-slices. Pass bh=0 is K-major (dense PE stream
        # consuming planes in production order); its PSUM eviction + output
        # DMA overlap pass bh=1. Pass bh=1 is o-major so each o-tile's
        # eviction + DMA trickle out during the remaining matmuls.
        ps = [
            [
                ps_pool.tile(
                    [128, 512], F32, tag=f"ps{o}_{bh}", name=f"ps{o}_{bh}"
                )
                for bh in range(2)
            ]
            for o in range(NCHUNK)
        ]
        NK = NCHUNK * NPLANES

        # PE warm-up: ~3.8us of zero matmuls with no data deps at kernel
        # start, so the HAM clock-gate reaches 8/8 (one full busy SHORT
        # window) before the first real matmul arrives at ~12us.
        wz = c3_pool.tile([128, 512], F16, tag="warmz")
        nc.gpsimd.memset(wz[:], 0.0)
        for _ in range(10):
            nc.tensor.matmul(
                ps[0][0][:64, :], wz[:, :64], wz[:], start=True, stop=True
            )

        def emit_copy_out(o, bh):
            # Copies alternate ACT/DVE; output DMA triggers alternate the
            # sync/scalar HWDGE rings so the tail's trigger issue (~0.6us
            # each on the issuing engine) parallelizes instead of queueing.
            ot = ost_pool.tile([128, 512], F32, tag=f"ot{o}_{bh}", name=f"ot{o}_{bh}")
            if o % 2 == 0:
                nc.scalar.activation(ot[:], ps[o][bh][:], AF.Copy)
            else:
                nc.vector.tensor_copy(ot[:], ps[o][bh][:])
            eng = nc.sync if o % 2 == 0 else nc.scalar
            eng.dma_start(
                out[o * 128 : (o + 1) * 128, bh * 512 : (bh + 1) * 512], ot[:]
            )

        for kk in range(NK):
            c, r = divmod(kk, NPLANES)
            rt = rho[0][c][r]
            for o in range(4):
                lhsT = c3_sb[c][:, r * O_FEAT + o * 128 : r * O_FEAT + (o + 1) * 128]
                nc.tensor.matmul(
                    ps[o][0][:], lhsT, rt[:], start=(kk == 0), stop=(kk == NK - 1)
                )
        for o in range(4):
            emit_copy_out(o, 0)
        KTAIL = NK - 4
        for kk in range(KTAIL):
            c, r = divmod(kk, NPLANES)
            rt = rho[1][c][r]
            for o in range(4):
                lhsT = c3_sb[c][:, r * O_FEAT + o * 128 : r * O_FEAT + (o + 1) * 128]
                nc.tensor.matmul(
                    ps[o][1][:], lhsT, rt[:], start=(kk == 0), stop=False
                )
        for o in range(4):
            for kk in range(KTAIL, NK):
                c, r = divmod(kk, NPLANES)
                lhsT = c3_sb[c][:, r * O_FEAT + o * 128 : r * O_FEAT + (o + 1) * 128]
                nc.tensor.matmul(
                    ps[o][1][:],
                    lhsT,
                    rho[1][c][r][:],
                    start=False,
                    stop=(kk == NK - 1),
                )
            emit_copy_out(o, 1)
    _split_all_waits(nc)
    return nc


# Basis-change: B3[j](t) = sum_r W[j,r] * plane_r(t), planes ordered
# [L4,L5,L6,L7, R4,R5,R6,R7]; binomial divided-difference weights /6.
_W6 = np.array(
    [
        [0, 0, 0, 0, 1, 0, 0, 0],
        [0, 0, 0, 0, -4, 1, 0, 0],
        [0, 0, 0, 0, 6, -4, 1, 0],
        [0, 0, 0, 0, -4, 6, -4, 1],
        [1, -4, 6, -4, 0, 0, 0, 0],
        [0, 1, -4, 6, 0, 0, 0, 0],
        [0, 0, 1, -4, 0, 0, 0, 0],
        [0, 0, 0, 1, 0, 0, 0, 0],
    ],
    dtype=np.float64,
)

_nc_cache: dict = {}


def _prepare(x: np.ndarray, coefficients: np.ndarray, grid: np.ndarray):
    x = np.asarray(x, dtype=np.float32)
    coefficients = np.asarray(coefficients, dtype=np.float32)
    grid = np.asarray(grid, dtype=np.float32)

    # Knot-coordinate transform t = (tanh(x) - grid[0]) / h (uniform grid).
    h = float(grid[-1] - grid[0]) / (len(grid) - 1)
    t_scale = 1.0 / h
    t_bias = -float(grid[0]) / h  # t = t_scale * xn + t_bias; here 2.5, 5.5

    key = (round(t_scale, 9), round(t_bias, 9))
    if key not in _nc_cache:
        _nc_cache[key] = _build_nc(t_scale, t_bias)
    nc = _nc_cache[key]

    # Host-side coefficient fold: C3[i, r, o] = sum_j coeff[i,o,j] * W[j,r] / 6,
    # with R-plane columns negated (device computes -R via min(w,0)*w^2).
    w = _W6 / 6.0
    w[:, 4:] *= -1.0
    # Interleave plane columns [L4,R4,L5,R5,...] to match production order.
    w = w[:, [0, 4, 1, 5, 2, 6, 3, 7]]
    c3f = np.einsum("ioj,jr->iro", coefficients.astype(np.float64), w)
    c3_arr = np.ascontiguousarray(
        c3f.reshape(NCHUNK, 128, NPLANES, O_FEAT)
        .reshape(NCHUNK, 128, NPLANES * O_FEAT)
        .astype(np.float16)
    )

    xt = np.ascontiguousarray(x.T.astype(np.float16))  # (512, 8192) f16
    in_maps = [
        {
            "xt": np.ascontiguousarray(xt[:, c * B_SHARD : (c + 1) * B_SHARD]),
            "c3": c3_arr,
        }
        for c in range(N_CORES)
    ]
    return nc, in_maps


def kernel(x: np.ndarray, coefficients: np.ndarray, grid: np.ndarray) -> np.ndarray:
    nc, in_maps = _prepare(x, coefficients, grid)
    res = run_bass_kernel_spmd(nc, in_maps, list(range(N_CORES)), trace=False)
    out_t = np.concatenate(
        [res.results[i]["out"] for i in range(N_CORES)], axis=1
    )  # (512, 8192)
    return np.ascontiguousarray(out_t.T).astype(np.float32)



# revision 1
# speedup vs baseline: 1.3178x; 1.3178x over previous
"""Trainium2 Bass kernel for the B-spline (KAN-style) layer.

Math: out[b,o] = sum_{i,k} B3_k(t(b,i)) * coeff[i,o,k], where t = tanh(x)
mapped to knot coordinates t = (tanh(x) - grid[0]) / h in (3, 8), and B3 are
cubic B-spline bases over uniform integer knots.

Key transformation: each cubic B-spline basis is an exact linear combination
of truncated cubes L_m = relu(t-m)^3 and R_m = relu(m-t)^3, m in {4,5,6,7}
(divided-difference / truncated-power identity, binomial weights [1,-4,6,-4,1]/6).
The 8x8 basis-change matrix is folded into the coefficient tensor on the host,
so the device only computes 8 "plane" tensors per input feature:
    w_m = 2.5*tanh(x) + (5.5 - m)   (DVE tensor_scalar)
    c_m = w_m^3                     (DVE: two tensor_mul)
    L_m = max(c_m, 0) ; Rn_m = min(c_m, 0) = -R_m   (DVE tensor_scalar max/min)
(the -R sign and the plane ordering are folded into the host coefficients).

Then a dense fp16 matmul: out[o,b] = sum_{(i,plane)} C3[(i,plane),o] * rho[(i,plane),b]
with contraction K = 512*8 = 4096, run on the PE at full bf16/fp16 rate.

Sharding: data-parallel over batch (8192 -> 8 x 1024); coefficients replicated.
Inputs are transposed on the host so the feature dim i lands on SBUF partitions
(making the contraction dim the partition dim for the matmul with no on-device
transposes); the output is produced as (o, b) and transposed back on the host.
"""

from contextlib import ExitStack

import numpy as np

import concourse.bass as bass
import concourse.mybir as mybir
import concourse.tile as tile
from concourse.bass_utils import run_bass_kernel_spmd
from concourse.vector_clock import ScopedClock

F32 = mybir.dt.float32
F16 = mybir.dt.float16

N_CORES = 8
B_FULL = 8192
B_SHARD = B_FULL // N_CORES  # 1024
I_FEAT = 512
O_FEAT = 512
NPLANES = 8
NCHUNK = I_FEAT // 128  # 4
ALU = mybir.AluOpType
AF = mybir.ActivationFunctionType

# ---------------------------------------------------------------------------
# Workaround for walrus "Too many sync wait commands" on the TileContext final
# Drain: spread the accumulated semaphore waits across single-wait nofuse NOPs
# on the sync engine, then emit a bare drain + the usual barrier/cleanup.
_MAXW = 1


def _patched_drain_and_barrier(self, tick_clock, wait_clock):
    nc = self.nc
    probe = nc.sync.nop(nofuse=True)
    wait_clock.add_sem_waits(probe.ins, ScopedClock({None: tick_clock.global_clock}))
    si = probe.ins.sync_info
    waits = list(si.on_wait) if si and si.on_wait else []
    if len(waits) > _MAXW:
        si.on_wait = waits[:_MAXW]
        rest = waits[_MAXW:]
        while rest:
            chunk, rest = rest[:_MAXW], rest[_MAXW:]
            n2 = nc.sync.nop(nofuse=True)
            s2 = n2.ins.sync_info
            if s2 is None:
                n2.ins.sync_info = mybir.SyncInfo(on_wait=chunk, on_update=[])
            else:
                s2.on_wait = chunk
    nc.sync.drain()
    nc.all_engine_barrier()
    assert self.sems is not None
    popped = nc._tile_sem_poison_stack.pop()
    assert popped is self._sem_poison
    nc.clear_and_free_semaphores(list(self.sems.allocated().values()))


tile.TileContext._drain_and_barrier = _patched_drain_and_barrier


def _split_all_waits(nc: bass.Bass) -> None:
    """This image's walrus rejects instructions carrying more than one sync
    wait. Hoist all but the last wait of each instruction onto fresh NoOps on
    the same engine immediately before it (in-order issue makes this
    equivalent, merely slightly stronger synchronization)."""
    cnt = 0
    for f in nc.m.functions:
        for bb in f.blocks:
            out = []
            changed = False
            for inst in bb.instructions:
                si = inst.sync_info
                waits = list(si.on_wait) if si and si.on_wait else []
                if len(waits) > 1:
                    changed = True
                    for w in waits[:-1]:
                        nop = mybir.InstNoOp(name=f"waitsplit-{cnt}", ins=[], outs=[])
                        cnt += 1
                        nop.engine = inst.engine
                        nop.sync_info = mybir.SyncInfo(on_wait=[w], on_update=[])
                        out.append(nop)
                    si.on_wait = [waits[-1]]
                out.append(inst)
            if changed:
                bb.instructions = out


# ---------------------------------------------------------------------------


def _build_nc(t_scale: float, t_bias: float) -> bass.Bass:
    """Build the per-core Bass program.

    Per-core I/O:
      xt : (512, 1024) f16   x^T shard (feature-major)
      c3 : (4, 128, 4096) f16  folded coefficients [chunk, part, plane*512+o]
      out: (512, 1024) f32   output (o, b) shard
    """
    nc = bass.Bass()
    xt = nc.declare_dram_parameter("xt", [I_FEAT, B_SHARD], F16, isOutput=False)
    c3 = nc.declare_dram_parameter(
        "c3", [NCHUNK, 128, NPLANES * O_FEAT], F16, isOutput=False
    )
    out = nc.declare_dram_parameter("out", [O_FEAT, B_SHARD], F32, isOutput=True)

    with tile.TileContext(nc) as tc, ExitStack() as ctx:
        c3_pool = ctx.enter_context(tc.tile_pool(name="c3", bufs=1))
        xin_pool = ctx.enter_context(tc.tile_pool(name="xin", bufs=2))
        xn_pool = ctx.enter_context(tc.tile_pool(name="xn", bufs=2))
        sq_pool = ctx.enter_context(tc.tile_pool(name="sq", bufs=3))
        aff_pool = ctx.enter_context(tc.tile_pool(name="aff", bufs=3))
        rho_pool = ctx.enter_context(tc.tile_pool(name="rho", bufs=1))
        ps_pool = ctx.enter_context(
            tc.tile_pool(name="ps", bufs=1, space=bass.MemorySpace.PSUM)
        )
        ost_pool = ctx.enter_context(tc.tile_pool(name="ost", bufs=1))

        BHALF = B_SHARD // 2  # 512

        # c3 chunks on the scalar engine's HWDGE ring (qActDynamicHW) so they
        # stream in parallel with the latency-critical xt loads on the sync
        # ring (qSPDynamicHW). Triggers cost ~0.6us each on ACT, which has
        # slack.
        # Dummy 1-column activation with no deps: hoists the ~2.7us ACT
        # table load to kernel start, off the tanh critical path.
        dummy = xn_pool.tile([128, 1], F16, tag="dummy")
        nc.gpsimd.memset(dummy[:], 0.0)
        nc.scalar.activation(dummy[:], dummy[:], AF.Tanh)

        c3_sb = []
        for c in range(NCHUNK):
            ct = c3_pool.tile([128, NPLANES * O_FEAT], F16, tag=f"c3_{c}")
            nc.scalar.dma_start(ct[:], c3[c])
            c3_sb.append(ct)

        # Elementwise plane production at half-batch granularity (FD=512) so
        # the first matmul's input chain is short; b-half-major order matches
        # the matmul passes. rho[bh][c][r] planes are interleaved
        # [L4, -R4, L5, -R5, ...] (sign + order folded into c3).
        rho = [[[None] * NPLANES for _ in range(NCHUNK)] for _ in range(2)]
        for bh in range(2):
            for c in range(NCHUNK):
                xt_sb = xin_pool.tile([128, BHALF], F16, tag="xt")
                nc.sync.dma_start(
                    xt_sb[:],
                    xt[c * 128 : (c + 1) * 128, bh * BHALF : (bh + 1) * BHALF],
                )
                xn = xn_pool.tile([128, BHALF], F16, tag="xn")
                nc.scalar.activation(xn[:], xt_sb[:], AF.Tanh)
                from contextlib import nullcontext

                from concourse.tile import add_dep_helper

                prev_rp = None
                for mi, m in enumerate((4, 5, 6, 7)):
                    beta = t_bias - m
                    # The very first plane gates the whole matmul stream: pin
                    # its chain to priority 0 so the scheduler doesn't
                    # interleave the next m's ops into it (~0.6us).
                    prio = (
                        tc.high_priority()
                        if (bh == 0 and c == 0 and mi == 0)
                        else nullcontext()
                    )
                    with prio:
                        a = aff_pool.tile([128, BHALF], F16, tag="a")
                        a_inst = nc.vector.tensor_scalar(
                            a[:], xn[:], t_scale, beta, ALU.mult, ALU.add
                        )
                        if bh == 0 and c == 0 and mi == 1 and prev_rp is not None:
                            # Order-only edge: keep the scheduler from
                            # interleaving m5's ops into the m4 chain that
                            # gates the whole matmul stream (~0.45us).
                            add_dep_helper(
                                a_inst.ins,
                                prev_rp.ins,
                                sync=False,
                                reason="first-plane chain priority",
                            )
                        # Square and cube on DVE: keeps the whole plane chain
                        # on one strict-FIFO engine (the scheduler once moved
                        # next-chunk tanh ahead of ACT squares, stalling 5us).
                        s = sq_pool.tile([128, BHALF], F16, tag="s")
                        nc.vector.tensor_mul(s[:], a[:], a[:])
                        cc = sq_pool.tile([128, BHALF], F16, tag="cube")
                        nc.vector.tensor_mul(cc[:], s[:], a[:])
                        lp = rho_pool.tile([128, BHALF], F16, tag=f"rho{bh}_{c}_{mi}")
                        nc.vector.tensor_scalar_max(lp[:], cc[:], 0.0)
                        rp = rho_pool.tile(
                            [128, BHALF], F16, tag=f"rho{bh}_{c}_{mi + 4}"
                        )
                        prev_rp = nc.vector.tensor_scalar_min(rp[:], cc[:], 0.0)
                    # Planes interleaved [L4,R4,L5,R5,...] in the K order so
                    # the matmul consumes them exactly in production order
                    # (the matching column permutation is folded into c3).
                    rho[bh][c][2 * mi] = lp
                    rho[bh][c][2 * mi + 1] = rp

        # Dense matmul: 8 PSUM tiles (o_chunk x b_half) accumulated over all
        # 32 (chunk, plane) K-slices. Pass bh=0 is K-major (dense PE stream
        # consuming planes in production order); its PSUM eviction + output
        # DMA overlap pass bh=1. Pass bh=1 is o-major so each o-tile's
        # eviction + DMA trickle out during the remaining matmuls.
        ps = [
            [
                ps_pool.tile(
                    [128, 512], F32, tag=f"ps{o}_{bh}", name=f"ps{o}_{bh}"
                )
                for bh in range(2)
            ]
            for o in range(NCHUNK)
        ]
        NK = NCHUNK * NPLANES

        # PE warm-up: ~3.8us of zero matmuls with no data deps at kernel
        # start, so the HAM clock-gate reaches 8/8 (one full busy SHORT
        # window) before the first real matmul arrives at ~12us.
        wz = c3_pool.tile([128, 512], F16, tag="warmz")
        nc.gpsimd.memset(wz[:], 0.0)
        for _ in range(10):
            nc.tensor.matmul(
                ps[0][0][:64, :], wz[:, :64], wz[:], start=True, stop=True
            )

        def emit_copy_out(o, bh):
            # Copies alternate ACT/DVE; output DMA triggers alternate the
            # sync/scalar HWDGE rings so the tail's trigger issue (~0.6us
            # each on the issuing engine) parallelizes instead of queueing.
            ot = ost_pool.tile([128, 512], F32, tag=f"ot{o}_{bh}", name=f"ot{o}_{bh}")
            if o % 2 == 0:
                nc.scalar.activation(ot[:], ps[o][bh][:], AF.Copy)
            else:
                nc.vector.tensor_copy(ot[:], ps[o][bh][:])
            eng = nc.sync if o % 2 == 0 else nc.scalar
            eng.dma_start(
                out[o * 128 : (o + 1) * 128, bh * 512 : (bh + 1) * 512], ot[:]
            )

        for kk in range(NK):
            c, r = divmod(kk, NPLANES)
            rt = rho[0][c][r]
            for o in range(4):
                lhsT = c3_sb[c][:, r * O_FEAT + o * 128 : r * O_FEAT + (o + 1) * 128]
                nc.tensor.matmul(
                    ps[o][0][:], lhsT, rt[:], start=(kk == 0), stop=(kk == NK - 1)
                )
        for o in range(4):
            emit_copy_out(o, 0)
        KTAIL = NK - 4
        for kk in range(KTAIL):
            c, r = divmod(kk, NPLANES)
            rt = rho[1][c][r]
            for o in range(4):
                lhsT = c3_sb[c][:, r * O_FEAT + o * 128 : r * O_FEAT + (o + 1) * 128]
                nc.tensor.matmul(
                    ps[o][1][:], lhsT, rt[:], start=(kk == 0), stop=False
                )
        for o in range(4):
            for kk in range(KTAIL, NK):
                c, r = divmod(kk, NPLANES)
                lhsT = c3_sb[c][:, r * O_FEAT + o * 128 : r * O_FEAT + (o + 1) * 128]
                nc.tensor.matmul(
                    ps[o][1][:],
                    lhsT,
                    rho[1][c][r][:],
                    start=False,
                    stop=(kk == NK - 1),
                )
            emit_copy_out(o, 1)
    _split_all_waits(nc)
    return nc


# Basis-change: B3[j](t) = sum_r W[j,r] * plane_r(t), planes ordered
# [L4,L5,L6,L7, R4,R5,R6,R7]; binomial divided-difference weights /6.
_W6 = np.array(
    [
        [0, 0, 0, 0, 1, 0, 0, 0],
        [0, 0, 0, 0, -4, 1, 0, 0],
        [0, 0, 0, 0, 6, -4, 1, 0],
        [0, 0, 0, 0, -4, 6, -4, 1],
        [1, -4, 6, -4, 0, 0, 0, 0],
        [0, 1, -4, 6, 0, 0, 0, 0],
        [0, 0, 1, -4, 0, 0, 0, 0],
        [0, 0, 0, 1, 0, 0, 0, 0],
    ],
    dtype=np.float64,
)

_nc_cache: dict = {}


def _prepare(x: np.ndarray, coefficients: np.ndarray, grid: np.ndarray):
    x = np.asarray(x, dtype=np.float32)
    coefficients = np.asarray(coefficients, dtype=np.float32)
    grid = np.asarray(grid, dtype=np.float32)

    # Knot-coordinate transform t = (tanh(x) - grid[0]) / h (uniform grid).
    h = float(grid[-1] - grid[0]) / (len(grid) - 1)
    t_scale = 1.0 / h
    t_bias = -float(grid[0]) / h  # t = t_scale * xn + t_bias; here 2.5, 5.5

    key = (round(t_scale, 9), round(t_bias, 9))
    if key not in _nc_cache:
        _nc_cache[key] = _build_nc(t_scale, t_bias)
    nc = _nc_cache[key]

    # Host-side coefficient fold: C3[i, r, o] = sum_j coeff[i,o,j] * W[j,r] / 6,
    # with R-plane columns negated (device computes -R via min(w,0)*w^2).
    w = _W6 / 6.0
    w[:, 4:] *= -1.0
    # Interleave plane columns [L4,R4,L5,R5,...] to match production order.
    w = w[:, [0, 4, 1, 5, 2, 6, 3, 7]]
    c3f = np.einsum("ioj,jr->iro", coefficients.astype(np.float64), w)
    c3_arr = np.ascontiguousarray(
        c3f.reshape(NCHUNK, 128, NPLANES, O_FEAT)
        .reshape(NCHUNK, 128, NPLANES * O_FEAT)
        .astype(np.float16)
    )

    xt = np.ascontiguousarray(x.T.astype(np.float16))  # (512, 8192) f16
    in_maps = [
        {
            "xt": np.ascontiguousarray(xt[:, c * B_SHARD : (c + 1) * B_SHARD]),
            "c3": c3_arr,
        }
        for c in range(N_CORES)
    ]
    return nc, in_maps


def kernel(x: np.ndarray, coefficients: np.ndarray, grid: np.ndarray) -> np.ndarray:
    nc, in_maps = _prepare(x, coefficients, grid)
    res = run_bass_kernel_spmd(nc, in_maps, list(range(N_CORES)), trace=False)
    out_t = np.concatenate(
        [res.results[i]["out"] for i in range(N_CORES)], axis=1
    )  # (512, 8192)
    return np.ascontiguousarray(out_t.T).astype(np.float32)

